# revision 1
# baseline (speedup 1.0000x reference)
"""Distance transform kernel for Trainium2 (8 NeuronCores, SPMD).

Computes, for each pixel (i,j) of a 128x128 grid, the min Euclidean distance
to any "boundary" pixel (feature_map > 0.5, pooled over batch/channel), and
broadcasts the result over the batch dimension.

Instead of the naive [H,W,H,W] pairwise min (268M candidate distances), uses
the exact separable two-phase Euclidean distance transform:
  phase 1: per-row 1D distance d1[h,j] = min_w |j-w| over boundary pixels of
           row h -- two hardware scans (state = min(state+1, pen[t])),
           forward and (via a reversed access pattern) backward.
  phase 2: dist^2[i,j] = min_h ( (i-h)^2 + d1[h,j]^2 ) -- min over h,
           exact for integer grids.

Sharding (halo): core c computes output rows i in [16c, 16c+16) and only
receives the HR-row neighborhood true-h in [16c-WIN/2, 16c-WIN/2+HR) of
the feature map (zero-padded outside the grid; zero rows have no boundary
pixels and yield sentinel distances that never win the min). In local
coordinates h' = h - (16c-WIN/2), every core runs the identical program
with the phase-2 window h' in [il, il+WIN) for local output row il -- this
window covers |h - i| <= WIN/2 - 1 = DMAX, so the result is exact whenever
the true distance field is everywhere <= DMAX (d(i,j) >= |i - h*| makes
max(dist) <= DMAX a sufficient host-side check). On failure the caller
falls back to a full-width program, keeping the kernel correct for any
input. For this problem's inputs (mask density ~255/256) distances are
~1-2, so the fast path always applies.

Output is batch-replicated, so no collectives are needed; the host gathers
the per-core [128,16] column blocks, transposes, and broadcasts over batch.
"""

import ml_dtypes
import numpy as np

import concourse.bacc as bacc
import concourse.masks as masks
import concourse.mybir as mybir
import concourse.tile as tile
from concourse.bass_utils import run_bass_kernel_spmd

H = 128          # grid height == width
B = 8            # batch
NCORES = 8
TI = H // NCORES  # output rows per core
HR = 24          # halo rows per core (windowed program)
WIN = 8          # phase-2 h-window per output row
DMAX = 3.0       # windowed result exact iff max distance <= DMAX

DT = mybir.dt.float32
SENTINEL = 1.0e4   # penalty for non-boundary pixels (>> max real distance)
SCAN_INIT = 1.0e9  # initial scan state
RED_INIT = 1.0e30  # pad value for log-step min fallback

import os as _os
USE_SCAN = _os.environ.get("K_USE_SCAN", "1") == "1"   # tensor_tensor_scan
USE_RSCAN = _os.environ.get("K_USE_RSCAN", "1") == "1"  # reversed-AP scan

_CACHE: dict = {}


def _logstep_prefix_min(nc, pool, src, rows, sign, tag):
    """Suffix (sign=+1) / prefix (sign=-1) min along the free dim via
    log-step shifted mins on a padded ping-pong buffer."""
    Alu = mybir.AluOpType
    pad = 64
    a = pool.tile([rows, H + pad], DT, tag=f"lsa{tag}")
    b = pool.tile([rows, H + pad], DT, tag=f"lsb{tag}")
    if sign > 0:
        data, padsl = slice(0, H), slice(H, H + pad)
    else:
        data, padsl = slice(pad, H + pad), slice(0, pad)
    nc.gpsimd.memset(a[:, padsl], RED_INIT)
    nc.gpsimd.memset(b[:, padsl], RED_INIT)
    nc.vector.tensor_copy(a[:, data], src)
    s = 1
    off = pad if sign < 0 else 0
    while s < H:
        sh = slice(off + sign * s, off + sign * s + H)
        nc.vector.tensor_tensor(b[:, data], a[:, data], a[:, sh], op=Alu.min)
        a, b = b, a
        s *= 2
    return a[:, data]


def _dmas(nc, pool, windowed, rows, fm_d, ib_d):
    """Issue the input DMAs. Emitted before the on-device constants so the
    Pool SWDGE descriptor generation isn't queued behind them."""
    hb = B // 2
    fdt = fm_d.dtype
    fm3 = fm_d.rearrange("b c h w -> h (b c) w")  # [rows, B, H]
    fmb = pool.tile([rows, hb, H], fdt, tag="fmb")
    nc.gpsimd.dma_start(fmb[:], fm3[:, hb:B])
    fma = pool.tile([rows, hb, H], fdt, tag="fma")
    nc.sync.dma_start(fma[:], fm3[:, 0:hb])
    ibx = None
    if not windowed:
        # ibias via the second HWDGE queue (ACT)
        ibx = pool.tile([H, 2 * TI], DT, tag="ibx")
        nc.scalar.dma_start(ibx[:], ib_d)
    return fma, fmb, ibx


def _body(nc, tc, pool, psumpool, windowed, rows, win, fm_d, ib_d, out_d,
          ident, iota_f, iotasq, psq, ones, sent, dmas=None):
    Alu = mybir.AluOpType
    if dmas is None:
        dmas = _dmas(nc, pool, windowed, rows, fm_d, ib_d)
    fma, fmb, ibx = dmas
    if not windowed:
        m2i = ibx[:, 0:TI]
        isq = ibx[:, TI:2 * TI]

    # union over batch: wide max tree, halves overlap the DMAs
    fdt = fma[:].dtype
    ma = pool.tile([rows, 2 * H], fdt, tag="ma")
    fma2 = fma[:].rearrange("p b w -> p (b w)")
    fmb2 = fmb[:].rearrange("p b w -> p (b w)")
    nc.vector.tensor_tensor(ma[:], fma2[:, 0:2 * H],
                            fma2[:, 2 * H:4 * H], op=Alu.max)
    mb = pool.tile([rows, 2 * H], fdt, tag="mb")
    nc.vector.tensor_tensor(mb[:], fmb2[:, 0:2 * H],
                            fmb2[:, 2 * H:4 * H], op=Alu.max)
    m2t = pool.tile([rows, 2 * H], fdt, tag="m2t")
    nc.vector.tensor_tensor(m2t[:], ma[:], mb[:], op=Alu.max)
    mx = pool.tile([rows, H], fdt, tag="mx")
    nc.vector.tensor_tensor(mx[:], m2t[:, 0:H], m2t[:, H:2 * H], op=Alu.max)

    # penalty: 0 where boundary, SENTINEL elsewhere. bf16 path: boundary
    # is mx >= 0.5 (truncated input); f32 path: boundary is mx > 0.5.
    pdt = fdt if windowed else DT
    pen = pool.tile([rows, H], pdt, tag="pen")
    nc.vector.tensor_scalar(out=pen[:], in0=mx[:], scalar1=0.5,
                            scalar2=sent[0:rows, 0:1],
                            op0=(Alu.is_lt if windowed else Alu.is_le),
                            op1=Alu.mult)

    # phase 1: 1D distance per row via hardware scans (state is fp32
    # internally; bf16 outputs are exact for integer distances <= 256)
    fsc = pool.tile([rows, H], pdt, tag="fsc")
    d1 = pool.tile([rows, H], pdt, tag="d1")
    if USE_SCAN:
        nc.vector.tensor_tensor_scan(fsc[:], ones[0:rows, :], pen[:],
                                     SCAN_INIT, op0=Alu.add, op1=Alu.min)
        if USE_RSCAN:
            bsc = pool.tile([rows, H], pdt, tag="bscr")
            nc.vector.tensor_tensor_scan(bsc[:], ones[0:rows, :],
                                         pen[:, ::-1], SCAN_INIT,
                                         op0=Alu.add, op1=Alu.min)
            nc.vector.tensor_tensor(d1[:], fsc[:], bsc[:, ::-1], op=Alu.min)
        else:
            v = pool.tile([rows, H], DT, tag="v")
            nc.vector.tensor_tensor(v[:], pen[:], iota_f[0:rows, :],
                                    op=Alu.add)
            vsf = _logstep_prefix_min(nc, pool, v[:], rows, +1, "s")
            bsc = pool.tile([rows, H], DT, tag="bsc")
            nc.vector.tensor_tensor(bsc[:], vsf, iota_f[0:rows, :],
                                    op=Alu.subtract)
            nc.vector.tensor_tensor(d1[:], fsc[:], bsc[:], op=Alu.min)
    else:
        u = pool.tile([rows, H], DT, tag="u")
        nc.vector.tensor_tensor(u[:], pen[:], iota_f[0:rows, :],
                                op=Alu.subtract)
        upf = _logstep_prefix_min(nc, pool, u[:], rows, -1, "p")
        nc.vector.tensor_tensor(fsc[:], upf, iota_f[0:rows, :], op=Alu.add)
        v = pool.tile([rows, H], DT, tag="v")
        nc.vector.tensor_tensor(v[:], pen[:], iota_f[0:rows, :], op=Alu.add)
        vsf = _logstep_prefix_min(nc, pool, v[:], rows, +1, "s")
        bsc = pool.tile([rows, H], DT, tag="bsc")
        nc.vector.tensor_tensor(bsc[:], vsf, iota_f[0:rows, :],
                                op=Alu.subtract)
        nc.vector.tensor_tensor(d1[:], fsc[:], bsc[:], op=Alu.min)

    # transpose d1 (PE, pass-through so PSUM dtype matches d1), square
    # it (ACT, PSUM->SBUF, converts to f32 -- exact for ints <= 256)
    pt = psumpool.tile([H, rows], pdt, tag="pt")
    nc.tensor.transpose(pt[:], d1[:], ident[:])
    t2 = pool.tile([H, rows], DT, tag="t2")  # d1[h,j]^2 at [j,h]
    nc.scalar.square(t2[:], pt[:])

    nd = 10                       # phase-2 output rows on DVE
    np_ = TI - nd                 # phase-2 output rows on Pool
    bigt = pool.tile([H, TI * win], DT, tag="bigt")
    biga = bigt[:, 0:nd * win]
    bigb = bigt[:, nd * win:TI * win]
    d2 = pool.tile([H, TI], DT, tag="d2")

    if windowed:
        # phase 2: cand[j, il, k] = d1T^2[j, il+k] + (k - WIN/2)^2; the
        # parabola row is il-independent in local coordinates, so ALL
        # output rows of an engine are one wide add over an overlapping
        # strided view of t2 (block step 1, inner step 1).
        import bass_rust
        t2ap = t2[:]

        def t2_blocks(first, count):
            return bass_rust.AP(
                t2ap.tensor, t2ap.offset + first,
                [list(t2ap.ap[0]), [1, count], [1, win]])

        nc.vector.tensor_tensor(
            biga.rearrange("p (a k) -> p a k", k=win),
            t2_blocks(0, nd),
            psq[:, 0:nd * win].rearrange("p (a k) -> p a k", k=win),
            op=Alu.add)
        nc.gpsimd.tensor_tensor(
            bigb.rearrange("p (a k) -> p a k", k=win),
            t2_blocks(nd, np_),
            psq[:, 0:np_ * win].rearrange("p (a k) -> p a k", k=win),
            op=Alu.add)
    else:
        # phase 2 via i-dependent scalars:
        # cand = (iota * -2i) + (d1T^2 + h^2); +i^2 added at the end
        t2h = pool.tile([H, rows], DT, tag="t2h")
        nc.vector.tensor_tensor(t2h[:], t2[:], iotasq[:, 0:rows], op=Alu.add)
        for il in range(nd):
            nc.vector.scalar_tensor_tensor(
                out=biga[:, il * win:(il + 1) * win], in0=iota_f[:, 0:win],
                scalar=m2i[:, il:il + 1], in1=t2h[:, 0:win],
                op0=Alu.mult, op1=Alu.add)
        for il in range(nd, TI):
            k = il - nd
            sl = slice(k * win, (k + 1) * win)
            nc.gpsimd.tensor_scalar(
                out=bigb[:, sl], in0=iota_f[:, 0:win],
                scalar1=m2i[:, il:il + 1], scalar2=None, op0=Alu.mult)
            nc.gpsimd.tensor_tensor(bigb[:, sl], bigb[:, sl],
                                    t2h[:, 0:win], op=Alu.add)

    nc.vector.tensor_reduce(
        d2[:, 0:nd], biga.rearrange("p (i h) -> p i h", h=win),
        axis=mybir.AxisListType.X, op=Alu.min)
    nc.vector.tensor_reduce(
        d2[:, nd:TI], bigb.rearrange("p (i h) -> p i h", h=win),
        axis=mybir.AxisListType.X, op=Alu.min)

    if not windowed:
        d2f = pool.tile([H, TI], DT, tag="d2f")
        nc.vector.tensor_tensor(d2f[:], d2[:], isq[:], op=Alu.add)
        d2 = d2f
    res = pool.tile([H, TI], DT, tag="res")
    nc.scalar.sqrt(res[:], d2[:])
    nc.sync.dma_start(out_d, res[:])


def _build_program(windowed: bool, repeat: int = 1, hw_loop_iters: int = 0):
    """One SPMD program. windowed=True: fm input is the per-core halo
    [B,1,HR,H] and phase 2 uses WIN-wide h-windows. windowed=False: fm is
    the full [B,1,H,H] image and phase 2 scans all 128 rows. repeat>1
    re-runs the whole body (incl. DMAs) for marginal-time measurement."""
    Alu = mybir.AluOpType
    rows = HR if windowed else H          # mask rows processed on this core
    win = WIN if windowed else H          # phase-2 candidate rows per output
    # windowed path ships the feature map as truncated bf16: the input is
    # only ever compared against 0.5 and trunc16(v) >= 0.5 <=> v > 0.5
    # (v == 0.5 exactly is host-guarded); bf16 gets the DVE 2x mode on the
    # max tree, the widest ops on the critical path.
    fdt = mybir.dt.bfloat16 if windowed else DT
    nc = bacc.Bacc("TRN2", target_bir_lowering=False, debug=False,
                   num_devices=NCORES)
    fm_d = nc.dram_tensor("fm", [B, 1, rows, H], fdt,
                          kind="ExternalInput").ap()
    ib_d = None
    if not windowed:
        # per-core side input: columns [0:TI] = -2*i, [TI:2TI] = i^2
        ib_d = nc.dram_tensor("ibias", [H, 2 * TI], DT,
                              kind="ExternalInput").ap()
    out_d = nc.dram_tensor("out", [H, TI], DT, kind="ExternalOutput").ap()

    with tile.TileContext(nc) as tc:
        with tc.tile_pool(name="main", bufs=1) as pool, \
             tc.tile_pool(name="psum", bufs=1, space="PSUM") as psumpool:

            dmas = None
            if not hw_loop_iters and repeat == 1:
                dmas = _dmas(nc, pool, windowed, rows, fm_d, ib_d)

            # constants built on device (during the first DMAs)
            cdt = mybir.dt.bfloat16 if windowed else DT
            ident = pool.tile([rows, rows], cdt, tag="ident")
            masks.make_identity(nc, ident[:])
            # sentinel via an early live Sqrt: makes the ACT func-table
            # pass load the sqrt set (which also contains Square) once,
            # instead of a mid-kernel 1.3us table switch before the final
            # sqrt. pen consumes it as a per-partition scalar.
            sent2 = pool.tile([H, 1], DT, tag="sent2")
            nc.gpsimd.memset(sent2[:], SENTINEL * SENTINEL)
            sent = pool.tile([H, 1], DT, tag="sent")
            nc.scalar.sqrt(sent[:], sent2[:])
            iota_f = iotasq = None
            if not (windowed and USE_SCAN and USE_RSCAN):
                iota_i = pool.tile([H, H], mybir.dt.int32, tag="iota_i")
                nc.gpsimd.iota(iota_i[:], pattern=[[1, H]], base=0,
                               channel_multiplier=0)
                iota_f = pool.tile([H, H], DT, tag="iota_f")
                nc.vector.tensor_copy(iota_f[:], iota_i[:])
                iotasq = pool.tile([H, H], DT, tag="iotasq")
                nc.scalar.square(iotasq[:], iota_f[:])
            if windowed:
                # psq[:, a*WIN + k] = (k - WIN/2)^2 for every block a: the
                # (i-h)^2 parabola is the same WIN-vector for every output
                # row in local coordinates, replicated TI times so phase 2
                # can consume it in one wide op per engine.
                psq_i = pool.tile([H, TI * WIN], mybir.dt.int32, tag="psq_i")
                nc.gpsimd.iota(psq_i[:], pattern=[[0, TI], [1, WIN]],
                               base=-WIN // 2, channel_multiplier=0)
                psq_f = pool.tile([H, TI * WIN], DT, tag="psq_f")
                nc.vector.tensor_copy(psq_f[:], psq_i[:])
                psq = pool.tile([H, TI * WIN], DT, tag="psq")
                nc.scalar.square(psq[:], psq_f[:])
            ones = pool.tile([rows, H], cdt, tag="ones")
            nc.gpsimd.memset(ones[:], 1.0)

            if hw_loop_iters:
                with tc.For_i(0, hw_loop_iters, 1):
                    _body(nc, tc, pool, psumpool, windowed, rows, win,
                          fm_d, ib_d, out_d, ident, iota_f, iotasq,
                          psq if windowed else None, ones, sent)
            else:
                for _rep in range(repeat):
                    _body(nc, tc, pool, psumpool, windowed, rows, win,
                          fm_d, ib_d, out_d, ident, iota_f, iotasq,
                          psq if windowed else None, ones, sent,
                          dmas=dmas if _rep == 0 else None)

    nc.compile()
    return nc


def _get_program(windowed: bool):
    key = "win" if windowed else "full"
    if key not in _CACHE:
        _CACHE[key] = _build_program(windowed)
    return _CACHE[key]


def _in_maps(feature_map: np.ndarray, windowed: bool):
    maps = []
    for c in range(NCORES):
        if windowed:
            # halo rows are true h in [16c-WIN/2, ...), zero-padded outside
            # the grid (zero rows have no boundary pixels). Shipped as
            # truncated bf16: v > 0.5 <=> trunc16(v) >= 0.5 for v != 0.5.
            lo = 16 * c - WIN // 2
            fm_c = np.zeros((B, 1, HR, H), np.float32)
            s, e = max(0, lo), min(H, lo + HR)
            fm_c[:, :, s - lo:e - lo, :] = feature_map[:, :, s:e, :]
            fm_bf = (np.ascontiguousarray(fm_c).view(np.uint32) >> 16) \
                .astype(np.uint16).view(ml_dtypes.bfloat16)
            maps.append({"fm": fm_bf})
        else:
            iv = np.arange(c * TI, (c + 1) * TI, dtype=np.float32)
            row = np.concatenate([-2.0 * iv, iv * iv])
            maps.append({
                "fm": np.ascontiguousarray(feature_map),
                "ibias": np.ascontiguousarray(
                    np.broadcast_to(row[None, :], (H, 2 * TI))),
            })
    return maps


def _run(feature_map, windowed, trace=False):
    nc = _get_program(windowed)
    out = run_bass_kernel_spmd(nc, _in_maps(feature_map, windowed),
                               list(range(NCORES)), trace=trace)
    _CACHE["last_result"] = out
    # per-core block c is [128(j), 16(i_local)] with i = 16c + i_local
    cols = np.concatenate([r["out"] for r in out.results], axis=1)
    return cols.T  # [i, j]


def kernel(feature_map: np.ndarray, _trace: bool = False):
    fm = np.ascontiguousarray(np.asarray(feature_map, dtype=np.float32))
    assert fm.shape == (B, 1, H, H), fm.shape
    if np.any(fm == np.float32(0.5)):
        # bf16-truncation trick needs v != 0.5 exactly; exact full program
        dist = _run(fm, windowed=False, trace=_trace)
        return np.ascontiguousarray(
            np.broadcast_to(dist[None, None], (B, 1, H, H))
            .astype(np.float32))
    dist = _run(fm, windowed=True, trace=_trace)
    if not np.all(dist <= DMAX + 0.01):  # margin for ACT sqrt rounding
        # windowed result not provably exact -> exact full-width program
        dist = _run(fm, windowed=False, trace=_trace)
    return np.ascontiguousarray(
        np.broadcast_to(dist[None, None], (B, 1, H, H)).astype(np.float32))



# revision 2
# speedup vs baseline: 3.6511x; 3.6511x over previous
"""Distance transform kernel for Trainium2 (8 NeuronCores, SPMD).

Computes, for each pixel (i,j) of a 128x128 grid, the min Euclidean distance
to any "boundary" pixel (feature_map > 0.5, pooled over batch/channel), and
broadcasts the result over the batch dimension.

Fast path: the mask density for this problem's input distribution is
1 - 2^-8 = 255/256 per pixel, so the true distance field is <= sqrt(2)
everywhere with probability ~1 - 4e-18.  A 3x3 min-plus stencil in squared
space is exact in that regime:

  pen(h,w)  = 0 if boundary else SENT
  A2(h,j)   = min_dw  pen(h, j+dw) + dw^2        dw in {-1,0,1}
  d2(i,j)   = min_dh  A2(i+dh, j)  + dh^2        dh in {-1,0,1}
  d         = sqrt(d2);  d2 in {0,1,2}  ->  d = min(d2, (sqrt2-1)*d2
                                                       + (2-sqrt2))  exactly

The pipeline runs entirely on the DVE (11 instructions per body): a 3-level
batch-union max tree (tensor_tensor gets the 2x bf16 mode; tensor_reduce
does not), penalty compare, two horizontal stencil ops on a sentinel-padded
tile, a 32x32 StreamTranspose (h<->w within 32-wide column blocks), two
vertical stencil ops on the block-transposed view, and a two-op exact sqrt.
No PE/ACT/Pool compute -> no cross-engine hops, no PSUM, and the result is
bit-exact vs the f32 reference.  The input DMA rides the SP HWDGE queue and
the output DMA the ACT queue: a DMA trigger holds its sequencer while its
waits are pending, so sharing one queue would stall the next body's input
prefetch behind this body's output drain.

Sharding: core c computes output rows i in [16c, 16c+16) from an 18-row
halo (true h in [16c-1, 16c+17), zero-padded outside the grid; zero rows
have no boundary pixels).  The host ships the halo as [18, 8, 128] bf16
([h', b, w], w contiguous, so the DMA is an 18-descriptor contiguous copy
and every max-tree view keeps the packed innermost dim the DVE 2x mode
needs; bf16 truncation keeps the 0.5 compare exact for v != 0.5, which is
host-guarded).  Output per core is [32, 4, 16] f32 with d(16c+i, 32k+a) at
[a, k, i]; the host de-interleaves and broadcasts over batch.  The output
is batch-replicated, so no collectives are needed.

Host-side guard: if any computed distance exceeds sqrt(2) (or any input is
exactly 0.5), rerun with the exact full-width program, keeping the kernel
correct for any input.
"""

import ml_dtypes
import numpy as np

import concourse.bacc as bacc
import concourse.masks as masks
import concourse.mybir as mybir
import concourse.tile as tile
from concourse.bass_utils import run_bass_kernel_spmd

H = 128          # grid height == width
B = 8            # batch
NCORES = 8
TI = H // NCORES  # output rows per core
HR = TI + 2      # halo rows per core (fast path): one extra row each side
DMAX = 1.4143    # fast-path result exact iff max distance <= sqrt(2)

DT = mybir.dt.float32
BF = mybir.dt.bfloat16
SENT = 1.0e4     # penalty for non-boundary pixels (>> max real distance)
SQRT2 = 1.41421356237309515

# ---- full-width fallback program constants (exact for any input) ----
WIN_FULL = H
SCAN_INIT = 1.0e9

_CACHE: dict = {}


# --------------------------------------------------------------------------
# fast path: 3x3 stencil, DVE-only
# --------------------------------------------------------------------------

def _body_fast(nc, pool, fm_d, out_d, pp, a2, tag=""):
    """One pipeline body.  pp is the [HR, H+2] penalty tile with sentinel
    pads in columns 0 and H+1 (set once at setup); a2 is a [32, H] tile
    whose rows HR..31 were set to sentinel once at setup (StreamTranspose
    needs a 32-multiple partition extent; the garbage rows land in
    never-read positions)."""
    Alu = mybir.AluOpType

    fmt = pool.tile([HR, B, H], BF, tag="fmt" + tag)
    nc.sync.dma_start(fmt[:], fm_d)
    fmf = fmt[:].rearrange("p b w -> p (b w)")

    # union over batch: max tree
    u1 = pool.tile([HR, 4 * H], BF, tag="u1" + tag)
    nc.vector.tensor_tensor(u1[:], fmf[:, 0:4 * H], fmf[:, 4 * H:8 * H],
                            op=Alu.max)
    u2 = pool.tile([HR, 2 * H], BF, tag="u2" + tag)
    nc.vector.tensor_tensor(u2[:], u1[:, 0:2 * H], u1[:, 2 * H:4 * H],
                            op=Alu.max)
    mx = pool.tile([HR, H], BF, tag="mx" + tag)
    nc.vector.tensor_tensor(mx[:], u2[:, 0:H], u2[:, H:2 * H], op=Alu.max)
    # penalty: 0 where boundary (mx >= 0.5 on truncated bf16), SENT else
    nc.vector.tensor_scalar(out=pp[:, 1:H + 1], in0=mx[:], scalar1=0.5,
                            scalar2=SENT, op0=Alu.is_lt, op1=Alu.mult)

    # horizontal pass: A2[h,j] = min(pen[h,j], min(pen[h,j-1],pen[h,j+1])+1)
    s = pool.tile([HR, H], BF, tag="s" + tag)
    nc.vector.tensor_tensor(s[:], pp[:, 0:H], pp[:, 2:H + 2], op=Alu.min)
    nc.vector.scalar_tensor_tensor(out=a2[0:HR, :], in0=s[:], scalar=1.0,
                                   in1=pp[:, 1:H + 1], op0=Alu.add,
                                   op1=Alu.min)

    # 32x32 block transpose: vt[a, 32k+b] = a2[b, 32k+a]
    vt = pool.tile([32, H], BF, tag="vt" + tag)
    nc.vector.transpose(vt[:], a2[:])
    v3 = vt[:].rearrange("p (k b) -> p k b", b=32)

    # vertical pass on the block-transposed view (h' is now the free dim)
    s2 = pool.tile([32, 4, TI], BF, tag="s2" + tag)
    nc.vector.tensor_tensor(s2[:], v3[:, :, 0:TI], v3[:, :, 2:TI + 2],
                            op=Alu.min)
    d2 = pool.tile([32, 4, TI], DT, tag="d2" + tag)
    nc.vector.scalar_tensor_tensor(out=d2[:], in0=s2[:], scalar=1.0,
                                   in1=v3[:, :, 1:TI + 1], op0=Alu.add,
                                   op1=Alu.min)

    # exact sqrt on {0,1,2}: d = min(d2, (sqrt2-1)*d2 + (2-sqrt2))
    d2f = d2[:].rearrange("p k i -> p (k i)")
    t = pool.tile([32, 4 * TI], DT, tag="t" + tag)
    nc.vector.tensor_scalar(out=t[:], in0=d2f, scalar1=SQRT2 - 1.0,
                            scalar2=2.0 - SQRT2, op0=Alu.mult, op1=Alu.add)
    res = pool.tile([32, 4 * TI], DT, tag="res" + tag)
    nc.vector.tensor_tensor(res[:], d2f, t[:], op=Alu.min)

    nc.scalar.dma_start(out_d, res[:])


def _build_fast(repeat: int = 1, hw_loop_iters: int = 0, unroll: int = 1,
                staggered: bool = False):
    """Fast-path program.  With hw_loop_iters the body block (`unroll`
    independent bodies, each with its own tiles so consecutive bodies
    pipeline across engines) runs under an on-device For_i;
    staggered=True uses Tile's staggered-reset loop (stage-local
    semaphore resets instead of a full back-edge barrier)."""
    nc = bacc.Bacc("TRN2", target_bir_lowering=False, debug=False,
                   num_devices=NCORES)
    fm_d = nc.dram_tensor("fm", [HR, B, H], BF, kind="ExternalInput").ap()
    out_d = nc.dram_tensor("out", [32, 4 * TI], DT,
                           kind="ExternalOutput").ap()

    with tile.TileContext(nc) as tc:
        with tc.tile_pool(name="main", bufs=1) as pool:
            # setup constants (once): sentinel pads never overwritten by
            # the loop bodies
            pps, a2s = [], []
            for u in range(unroll):
                pp = pool.tile([HR, H + 2], BF, tag=f"pp{u}")
                nc.vector.memset(pp[:, 0:1], SENT)
                nc.vector.memset(pp[:, H + 1:H + 2], SENT)
                pps.append(pp)
                # whole-tile memset (partition ranges must start
                # 32-aligned); bodies overwrite rows 0..HR-1
                a2 = pool.tile([32, H], BF, tag=f"a2{u}")
                nc.vector.memset(a2[:], SENT)
                a2s.append(a2)

            if hw_loop_iters:
                with tc.For_i(0, hw_loop_iters, 1, staggered_reset=staggered):
                    for u in range(unroll):
                        _body_fast(nc, pool, fm_d, out_d, pps[u], a2s[u],
                                   tag=str(u))
            else:
                for r in range(repeat):
                    u = r % unroll
                    _body_fast(nc, pool, fm_d, out_d, pps[u], a2s[u],
                               tag=str(u))

    nc.compile()
    return nc


# --------------------------------------------------------------------------
# exact full-width fallback (any input): separable two-phase transform
# --------------------------------------------------------------------------

def _body_full(nc, pool, psumpool, fm_d, ib_d, out_d, ident, iota_f, iotasq,
               ones, sent):
    Alu = mybir.AluOpType
    rows = H
    hb = B // 2
    fm3 = fm_d.rearrange("b c h w -> h (b c) w")  # [rows, B, H]
    fmb = pool.tile([rows, hb, H], DT, tag="fmb")
    nc.gpsimd.dma_start(fmb[:], fm3[:, hb:B])
    fma = pool.tile([rows, hb, H], DT, tag="fma")
    nc.sync.dma_start(fma[:], fm3[:, 0:hb])
    ibx = pool.tile([H, 2 * TI], DT, tag="ibx")
    nc.scalar.dma_start(ibx[:], ib_d)
    m2i = ibx[:, 0:TI]
    isq = ibx[:, TI:2 * TI]

    # union over batch: wide max tree
    ma = pool.tile([rows, 2 * H], DT, tag="ma")
    fma2 = fma[:].rearrange("p b w -> p (b w)")
    fmb2 = fmb[:].rearrange("p b w -> p (b w)")
    nc.vector.tensor_tensor(ma[:], fma2[:, 0:2 * H],
                            fma2[:, 2 * H:4 * H], op=Alu.max)
    mb = pool.tile([rows, 2 * H], DT, tag="mb")
    nc.vector.tensor_tensor(mb[:], fmb2[:, 0:2 * H],
                            fmb2[:, 2 * H:4 * H], op=Alu.max)
    m2t = pool.tile([rows, 2 * H], DT, tag="m2t")
    nc.vector.tensor_tensor(m2t[:], ma[:], mb[:], op=Alu.max)
    mx = pool.tile([rows, H], DT, tag="mx")
    nc.vector.tensor_tensor(mx[:], m2t[:, 0:H], m2t[:, H:2 * H], op=Alu.max)

    # penalty: 0 where boundary (mx > 0.5 in f32), SENTINEL elsewhere
    pen = pool.tile([rows, H], DT, tag="pen")
    nc.vector.tensor_scalar(out=pen[:], in0=mx[:], scalar1=0.5,
                            scalar2=sent[0:rows, 0:1],
                            op0=Alu.is_le, op1=Alu.mult)

    # phase 1: 1D distance per row via hardware scans
    fsc = pool.tile([rows, H], DT, tag="fsc")
    d1 = pool.tile([rows, H], DT, tag="d1")
    nc.vector.tensor_tensor_scan(fsc[:], ones[0:rows, :], pen[:],
                                 SCAN_INIT, op0=Alu.add, op1=Alu.min)
    bsc = pool.tile([rows, H], DT, tag="bscr")
    nc.vector.tensor_tensor_scan(bsc[:], ones[0:rows, :],
                                 pen[:, ::-1], SCAN_INIT,
                                 op0=Alu.add, op1=Alu.min)
    nc.vector.tensor_tensor(d1[:], fsc[:], bsc[:, ::-1], op=Alu.min)

    # transpose d1 (PE), square it (ACT, PSUM->SBUF)
    pt = psumpool.tile([H, rows], DT, tag="pt")
    nc.tensor.transpose(pt[:], d1[:], ident[:])
    t2 = pool.tile([H, rows], DT, tag="t2")  # d1[h,j]^2 at [j,h]
    nc.scalar.square(t2[:], pt[:])

    # phase 2 via i-dependent scalars:
    # cand = (iota * -2i) + (d1T^2 + h^2); +i^2 added at the end
    nd = 10
    win = WIN_FULL
    bigt = pool.tile([H, TI * win], DT, tag="bigt")
    biga = bigt[:, 0:nd * win]
    bigb = bigt[:, nd * win:TI * win]
    d2 = pool.tile([H, TI], DT, tag="d2")
    t2h = pool.tile([H, rows], DT, tag="t2h")
    nc.vector.tensor_tensor(t2h[:], t2[:], iotasq[:, 0:rows], op=Alu.add)
    for il in range(nd):
        nc.vector.scalar_tensor_tensor(
            out=biga[:, il * win:(il + 1) * win], in0=iota_f[:, 0:win],
            scalar=m2i[:, il:il + 1], in1=t2h[:, 0:win],
            op0=Alu.mult, op1=Alu.add)
    for il in range(nd, TI):
        k = il - nd
        sl = slice(k * win, (k + 1) * win)
        nc.gpsimd.tensor_scalar(
            out=bigb[:, sl], in0=iota_f[:, 0:win],
            scalar1=m2i[:, il:il + 1], scalar2=None, op0=Alu.mult)
        nc.gpsimd.tensor_tensor(bigb[:, sl], bigb[:, sl],
                                t2h[:, 0:win], op=Alu.add)

    nc.vector.tensor_reduce(
        d2[:, 0:nd], biga.rearrange("p (i h) -> p i h", h=win),
        axis=mybir.AxisListType.X, op=Alu.min)
    nc.vector.tensor_reduce(
        d2[:, nd:TI], bigb.rearrange("p (i h) -> p i h", h=win),
        axis=mybir.AxisListType.X, op=Alu.min)

    d2f = pool.tile([H, TI], DT, tag="d2f")
    nc.vector.tensor_tensor(d2f[:], d2[:], isq[:], op=Alu.add)
    res = pool.tile([H, TI], DT, tag="res")
    nc.scalar.sqrt(res[:], d2f[:])
    nc.sync.dma_start(out_d, res[:])


def _build_full():
    nc = bacc.Bacc("TRN2", target_bir_lowering=False, debug=False,
                   num_devices=NCORES)
    fm_d = nc.dram_tensor("fm", [B, 1, H, H], DT, kind="ExternalInput").ap()
    # per-core side input: columns [0:TI] = -2*i, [TI:2TI] = i^2
    ib_d = nc.dram_tensor("ibias", [H, 2 * TI], DT,
                          kind="ExternalInput").ap()
    out_d = nc.dram_tensor("out", [H, TI], DT, kind="ExternalOutput").ap()

    with tile.TileContext(nc) as tc:
        with tc.tile_pool(name="main", bufs=1) as pool, \
             tc.tile_pool(name="psum", bufs=1, space="PSUM") as psumpool:
            ident = pool.tile([H, H], DT, tag="ident")
            masks.make_identity(nc, ident[:])
            sent2 = pool.tile([H, 1], DT, tag="sent2")
            nc.gpsimd.memset(sent2[:], SENT * SENT)
            sent = pool.tile([H, 1], DT, tag="sent")
            nc.scalar.sqrt(sent[:], sent2[:])
            iota_i = pool.tile([H, H], mybir.dt.int32, tag="iota_i")
            nc.gpsimd.iota(iota_i[:], pattern=[[1, H]], base=0,
                           channel_multiplier=0)
            iota_f = pool.tile([H, H], DT, tag="iota_f")
            nc.vector.tensor_copy(iota_f[:], iota_i[:])
            iotasq = pool.tile([H, H], DT, tag="iotasq")
            nc.scalar.square(iotasq[:], iota_f[:])
            ones = pool.tile([H, H], DT, tag="ones")
            nc.gpsimd.memset(ones[:], 1.0)

            _body_full(nc, pool, psumpool, fm_d, ib_d, out_d,
                       ident, iota_f, iotasq, ones, sent)

    nc.compile()
    return nc


# --------------------------------------------------------------------------
# host glue
# --------------------------------------------------------------------------

def _build_program(windowed: bool, repeat: int = 1, hw_loop_iters: int = 0,
                   unroll: int = 1, staggered: bool = False):
    if windowed:
        return _build_fast(repeat=repeat, hw_loop_iters=hw_loop_iters,
                           unroll=unroll, staggered=staggered)
    return _build_full()


def _get_program(windowed: bool):
    key = "win" if windowed else "full"
    if key not in _CACHE:
        _CACHE[key] = _build_program(windowed)
    return _CACHE[key]


def _to_bf16_trunc(a: np.ndarray) -> np.ndarray:
    """Truncate f32 -> bf16 (drop low mantissa bits).  Preserves the 0.5
    compare exactly: trunc16(v) >= 0.5  <=>  v >= 0.5."""
    return (np.ascontiguousarray(a).view(np.uint32) >> 16) \
        .astype(np.uint16).view(ml_dtypes.bfloat16)


def _in_maps(feature_map: np.ndarray, windowed: bool):
    maps = []
    for c in range(NCORES):
        if windowed:
            # halo rows are true h in [16c-1, 16c+17), zero-padded outside
            # the grid; layout [h', b, w] (w contiguous for the DVE 2x mode)
            lo = TI * c - 1
            fm_c = np.zeros((HR, B, H), np.float32)
            s, e = max(0, lo), min(H, lo + HR)
            fm_c[s - lo:e - lo] = feature_map[:, 0, s:e, :].transpose(1, 0, 2)
            maps.append({"fm": _to_bf16_trunc(fm_c)})
        else:
            iv = np.arange(c * TI, (c + 1) * TI, dtype=np.float32)
            row = np.concatenate([-2.0 * iv, iv * iv])
            maps.append({
                "fm": np.ascontiguousarray(feature_map),
                "ibias": np.ascontiguousarray(
                    np.broadcast_to(row[None, :], (H, 2 * TI))),
            })
    return maps


def _assemble_fast(results):
    """Per-core block c is [32(a), 4(k), 16(i)] holding d(16c+i, 32k+a)."""
    dist = np.empty((H, H), np.float32)
    for c, r in enumerate(results):
        blk = r["out"].reshape(32, 4, TI)
        dist[TI * c:TI * (c + 1), :] = blk.transpose(2, 1, 0).reshape(TI, H)
    return dist


def _run(feature_map, windowed, trace=False):
    nc = _get_program(windowed)
    out = run_bass_kernel_spmd(nc, _in_maps(feature_map, windowed),
                               list(range(NCORES)), trace=trace)
    _CACHE["last_result"] = out
    if windowed:
        return _assemble_fast(out.results)
    # per-core block c is [128(j), 16(i_local)] with i = 16c + i_local
    cols = np.concatenate([r["out"] for r in out.results], axis=1)
    return cols.T  # [i, j]


def kernel(feature_map: np.ndarray, _trace: bool = False):
    fm = np.ascontiguousarray(np.asarray(feature_map, dtype=np.float32))
    assert fm.shape == (B, 1, H, H), fm.shape
    if np.any(fm == np.float32(0.5)):
        # bf16-truncation trick needs v != 0.5 exactly; exact full program
        dist = _run(fm, windowed=False, trace=_trace)
        return np.ascontiguousarray(
            np.broadcast_to(dist[None, None], (B, 1, H, H))
            .astype(np.float32))
    dist = _run(fm, windowed=True, trace=_trace)
    if not np.all(dist <= DMAX):
        # fast-path result not provably exact -> exact full-width program
        dist = _run(fm, windowed=False, trace=_trace)
    return np.ascontiguousarray(
        np.broadcast_to(dist[None, None], (B, 1, H, H)).astype(np.float32))
